# revision 4
# baseline (speedup 1.0000x reference)
"""Distributed FlashRotarySelfAttention kernel for 8 TRN2 NeuronCores.

Reference computation (per nn_FlashRotarySelfAttention):
  qkv = x @ Wqkv;  k, q, v = split(qkv, 3)  [k first!]
  k, q = rope(k), rope(q)
  out = causal_softmax(q k^T / sqrt(Dh)) @ v
  return out @ Wproj

Sharding: tensor-parallel over heads. Core i owns heads {2i, 2i+1}:
  - column-parallel Wqkv (k|q|v columns of its 2 heads)
  - attention fully local per (batch, head)
  - AllGather of per-core attention outputs (transposed, c-major)
  - column-parallel Wproj: each core computes 256 output channels
Host concatenates + transposes the per-core outputs.

All matmuls run in bf16 with fp32 PSUM accumulation. Softmax skips the
max-subtraction (scores are O(10) here, exp is safe in fp32).
"""

import numpy as np
import ml_dtypes

import concourse.bacc as bacc
import concourse.mybir as mybir
import concourse.tile as tile
from concourse.bass_utils import run_bass_kernel_spmd

# Problem shapes (hardcoded per contest rules).
B, S, C, H = 2, 2048, 2048, 16
Dh = C // H                      # 128
BS = B * S                       # 4096
N_CORES = 8
H_LOC = H // N_CORES             # 2 heads per core
W_LOC = 3 * H_LOC * Dh           # 768 local qkv columns
CO_LOC = C // N_CORES            # 256 output channels per core
ROPE_THETA = 10000.0
SCALE = float(Dh) ** -0.5

F32 = mybir.dt.float32
BF16 = mybir.dt.bfloat16

P = 128            # partitions
QCH = 512          # q-chunk (matmul free dim)
N_SC = BS // QCH   # 8 s-chunks over B*S
N_CC = C // P      # 16 contraction chunks
N_QC = S // QCH    # 4 q-chunks per batch
N_KT = S // P      # 16 k-tiles per batch


def _host_constants():
    """Input-independent tables computed on host (compile-time constants)."""
    half = Dh // 2
    inv_freq = 1.0 / (ROPE_THETA ** (np.arange(0, half, dtype=np.float64) / half))
    ang = np.arange(S, dtype=np.float64)[None, :] * inv_freq[:, None]   # [64, S]
    cos_t = np.cos(ang).astype(np.float32)
    sin_t = np.sin(ang).astype(np.float32)
    # Causal 0/1 masks for diagonal score tiles, scoresT layout [k_local, q_local].
    # Tile j (k-tile index j within the q-chunk): keep iff q_local >= 128*j + k_local.
    kk = np.arange(P)[:, None]
    qq = np.arange(QCH)[None, :]
    masks = np.stack(
        [(qq >= P * j + kk) for j in range(4)], axis=0
    ).astype(ml_dtypes.bfloat16)                                        # [4, 128, 512]
    ident = np.eye(P, dtype=ml_dtypes.bfloat16)
    ones = np.ones((P, P), dtype=ml_dtypes.bfloat16)
    return cos_t, sin_t, masks, ident, ones


def build_nc():
    nc = bacc.Bacc(None, num_devices=N_CORES)

    x_in = nc.declare_dram_parameter("x", [BS, C], F32, isOutput=False)
    wqkv_in = nc.declare_dram_parameter("wqkv", [C, W_LOC], F32, isOutput=False)
    wproj_in = nc.declare_dram_parameter("wproj", [C, CO_LOC], F32, isOutput=False)
    cos_in = nc.declare_dram_parameter("cos_t", [Dh // 2, S], F32, isOutput=False)
    sin_in = nc.declare_dram_parameter("sin_t", [Dh // 2, S], F32, isOutput=False)
    masks_in = nc.declare_dram_parameter("masks", [4, P, QCH], BF16, isOutput=False)
    ident_in = nc.declare_dram_parameter("ident", [P, P], BF16, isOutput=False)
    ones_in = nc.declare_dram_parameter("ones", [P, P], BF16, isOutput=False)
    out_ext = nc.declare_dram_parameter("outT", [CO_LOC, BS], F32, isOutput=True)

    from contextlib import ExitStack

    with tile.TileContext(nc) as tc, ExitStack() as ctx:
        consts = ctx.enter_context(tc.tile_pool(name="consts", bufs=1))
        qkvp = ctx.enter_context(tc.tile_pool(name="qkvp", bufs=1))
        xt_pool = ctx.enter_context(tc.tile_pool(name="xt", bufs=2))
        rope_pool = ctx.enter_context(tc.tile_pool(name="rope", bufs=8))
        probs_pool = ctx.enter_context(tc.tile_pool(name="probs", bufs=4))
        vtmp_pool = ctx.enter_context(tc.tile_pool(name="vtmp", bufs=2))
        attn_pool = ctx.enter_context(tc.tile_pool(name="attn", bufs=2))
        gt_pool = ctx.enter_context(tc.tile_pool(name="gt", bufs=2))
        outp_pool = ctx.enter_context(tc.tile_pool(name="outp", bufs=2))
        dram = ctx.enter_context(tc.tile_pool(name="dram", bufs=1, space="DRAM"))
        mmps = ctx.enter_context(tc.tile_pool(name="mmps", bufs=2, space="PSUM"))
        ops_pool = ctx.enter_context(tc.tile_pool(name="ops", bufs=2, space="PSUM"))
        dps_pool = ctx.enter_context(tc.tile_pool(name="dps", bufs=2, space="PSUM"))
        vtps = ctx.enter_context(tc.tile_pool(name="vtps", bufs=2, space="PSUM"))
        if True:

            # ---- Phase 0: constants / weights to SBUF --------------------
            wqkv_sb = consts.tile([P, N_CC, W_LOC], BF16)
            nc.gpsimd.dma_start(
                wqkv_sb[:], wqkv_in.rearrange("(o p) w -> p o w", p=P)
            )
            wproj_sb = consts.tile([P, N_CC, CO_LOC], BF16)
            nc.gpsimd.dma_start(
                wproj_sb[:], wproj_in.rearrange("(o p) w -> p o w", p=P)
            )
            cos_sb = consts.tile([Dh // 2, S], F32)
            nc.sync.dma_start(cos_sb[:], cos_in[:])
            sin_sb = consts.tile([Dh // 2, S], F32)
            nc.sync.dma_start(sin_sb[:], sin_in[:])
            masks_sb = consts.tile([P, 4, QCH], BF16)
            nc.sync.dma_start(masks_sb[:], masks_in.rearrange("j p q -> p j q"))
            ident_sb = consts.tile([P, P], BF16)
            nc.sync.dma_start(ident_sb[:], ident_in[:])
            ones_sb = consts.tile([P, P], BF16)
            nc.sync.dma_start(ones_sb[:], ones_in[:])

            # Resident activations: d-major q/k, k-major v. bh = h_local*2 + b
            q_sb = qkvp.tile([P, 2 * H_LOC, S], BF16)
            k_sb = qkvp.tile([P, 2 * H_LOC, S], BF16)
            v_sb = qkvp.tile([P, 2 * H_LOC, N_KT, Dh], BF16)

            # ---- Phase 1+2: x cast to bf16, transpose-load, QKV ----------
            x_chunks = [dram.tile([QCH, C], BF16, name=f"xch{j}") for j in range(N_SC)]
            for sc in range(N_SC):
                # DRAM->DRAM inline f32->bf16 cast (SWDGE)
                nc.gpsimd.dma_start(
                    x_chunks[sc][:], x_in[sc * QCH:(sc + 1) * QCH, :]
                )

            for sc in range(N_SC):
                b = sc // N_QC
                s0 = (sc % N_QC) * QCH       # position offset within batch
                cos_c = cos_sb[:, s0:s0 + QCH]
                sin_c = sin_sb[:, s0:s0 + QCH]
                # x^T tile [c_in(128, o), s(512)] via XBAR transpose
                xt = xt_pool.tile([P, N_CC, QCH], BF16)
                nc.sync.dma_start_transpose(xt[:], x_chunks[sc][:])

                for ct in range(6):
                    ps = mmps.tile([P, QCH], F32, tag="mm")
                    for cc in range(N_CC):
                        nc.tensor.matmul(
                            ps[:],
                            lhsT=wqkv_sb[:, cc, ct * P:(ct + 1) * P],
                            rhs=xt[:, cc, :],
                            start=(cc == 0),
                            stop=(cc == N_CC - 1),
                        )
                    if ct < 4:
                        # k (ct 0,1) and q (ct 2,3): RoPE -> bf16 resident
                        hl = ct % 2
                        dst = k_sb if ct < 2 else q_sb
                        bh = hl * 2 + b
                        lo = ps[0:64, :]
                        hi = ps[64:128, :]
                        t1 = rope_pool.tile([64, QCH], F32, tag="rt")
                        t2 = rope_pool.tile([64, QCH], F32, tag="rt")
                        t3 = rope_pool.tile([64, QCH], F32, tag="rt")
                        t4 = rope_pool.tile([64, QCH], F32, tag="rt")
                        nc.any.tensor_tensor(t1[:], lo, cos_c, mybir.AluOpType.mult)
                        nc.any.tensor_tensor(t2[:], hi, sin_c, mybir.AluOpType.mult)
                        nc.any.tensor_tensor(
                            dst[0:64, bh, s0:s0 + QCH],
                            t1[:], t2[:], mybir.AluOpType.subtract,
                        )
                        nc.any.tensor_tensor(t3[:], hi, cos_c, mybir.AluOpType.mult)
                        nc.any.tensor_tensor(t4[:], lo, sin_c, mybir.AluOpType.mult)
                        nc.any.tensor_tensor(
                            dst[64:128, bh, s0:s0 + QCH],
                            t3[:], t4[:], mybir.AluOpType.add,
                        )
                    else:
                        # v (ct 4,5): cast to bf16, PE-transpose to k-major
                        hl = ct - 4
                        bh = hl * 2 + b
                        vt = vtmp_pool.tile([P, QCH], BF16)
                        nc.vector.tensor_copy(vt[:], ps[:])
                        for blk in range(QCH // P):
                            pt = vtps.tile([P, P], BF16)
                            nc.tensor.transpose(pt[:], vt[:, blk * P:(blk + 1) * P],
                                                ident_sb[:])
                            st = (sc % N_QC) * (QCH // P) + blk
                            nc.vector.tensor_copy(v_sb[:, bh, st, :], pt[:])

            # ---- Phase 3: attention per (b, h_local) ---------------------
            attn_dram = dram.tile([H_LOC * Dh, BS], BF16)
            for hl in range(H_LOC):
                for b in range(B):
                    bh = hl * 2 + b
                    for qc in range(N_QC):
                        n_kt = (QCH // P) * (qc + 1)
                        po = ops_pool.tile([P, QCH], F32, tag="po")
                        pd = dps_pool.tile([P, QCH], F32, tag="pd")
                        for kt in range(n_kt):
                            pscore = mmps.tile([P, QCH], F32, tag="mm")
                            nc.tensor.matmul(
                                pscore[:],
                                lhsT=k_sb[:, bh, kt * P:(kt + 1) * P],
                                rhs=q_sb[:, bh, qc * QCH:(qc + 1) * QCH],
                                start=True, stop=True,
                            )
                            pr = probs_pool.tile([P, QCH], BF16, tag="pr")
                            nc.scalar.activation(
                                pr[:], pscore[:],
                                mybir.ActivationFunctionType.Exp,
                                scale=SCALE,
                            )
                            j = kt - (QCH // P) * qc
                            if j >= 0:
                                nc.vector.tensor_tensor(
                                    pr[:], pr[:], masks_sb[:, j, :],
                                    mybir.AluOpType.mult,
                                )
                            nc.tensor.matmul(
                                po[:], lhsT=v_sb[:, bh, kt, :], rhs=pr[:],
                                start=(kt == 0), stop=(kt == n_kt - 1),
                            )
                            nc.tensor.matmul(
                                pd[:], lhsT=ones_sb[:], rhs=pr[:],
                                start=(kt == 0), stop=(kt == n_kt - 1),
                            )
                        recip = attn_pool.tile([P, QCH], F32, tag="rec")
                        nc.vector.reciprocal(recip[:], pd[:])
                        at = attn_pool.tile([P, QCH], BF16, tag="at")
                        nc.vector.tensor_tensor(
                            at[:], po[:], recip[:], mybir.AluOpType.mult
                        )
                        nc.sync.dma_start(
                            attn_dram[hl * Dh:(hl + 1) * Dh,
                                      b * S + qc * QCH:b * S + (qc + 1) * QCH],
                            at[:],
                        )

            # ---- Phase 4: AllGather heads across cores -------------------
            gathered = dram.tile([C, BS], BF16)
            nc.gpsimd.collective_compute(
                "AllGather",
                mybir.AluOpType.bypass,
                replica_groups=[list(range(N_CORES))],
                ins=[attn_dram[:].opt()],
                outs=[gathered[:].opt()],
            )

            # ---- Phase 5: output projection (column-parallel) ------------
            for sc in range(N_SC):
                gt = gt_pool.tile([P, N_CC, QCH], BF16)
                nc.sync.dma_start(
                    gt[:],
                    gathered[:, sc * QCH:(sc + 1) * QCH].rearrange(
                        "(o p) q -> p o q", p=P
                    ),
                )
                for ct in range(CO_LOC // P):
                    ps = mmps.tile([P, QCH], F32, tag="mm")
                    for cc in range(N_CC):
                        nc.tensor.matmul(
                            ps[:],
                            lhsT=wproj_sb[:, cc, ct * P:(ct + 1) * P],
                            rhs=gt[:, cc, :],
                            start=(cc == 0),
                            stop=(cc == N_CC - 1),
                        )
                    ot = outp_pool.tile([P, QCH], F32)
                    nc.vector.tensor_copy(ot[:], ps[:])
                    nc.sync.dma_start(
                        out_ext[ct * P:(ct + 1) * P, sc * QCH:(sc + 1) * QCH],
                        ot[:],
                    )

    nc.finalize()
    return nc


_NC_CACHE = None


def _get_nc():
    global _NC_CACHE
    if _NC_CACHE is None:
        _NC_CACHE = build_nc()
    return _NC_CACHE


def make_in_maps(x, Wqkv, Wproj):
    """Shard the full inputs across the 8 cores (host side)."""
    x2 = np.ascontiguousarray(np.asarray(x, dtype=np.float32).reshape(BS, C))
    Wqkv = np.asarray(Wqkv, dtype=np.float32)
    Wproj = np.asarray(Wproj, dtype=np.float32)
    cos_t, sin_t, masks, ident, ones = _host_constants()
    in_maps = []
    for i in range(N_CORES):
        h0 = H_LOC * i
        cols = []
        for part in range(3):  # k, q, v blocks (k first per reference)
            base = part * C + h0 * Dh
            cols.append(Wqkv[:, base:base + H_LOC * Dh])
        wqkv_loc = np.ascontiguousarray(np.concatenate(cols, axis=1))
        wproj_loc = np.ascontiguousarray(Wproj[:, i * CO_LOC:(i + 1) * CO_LOC])
        in_maps.append({
            "x": x2,
            "wqkv": wqkv_loc,
            "wproj": wproj_loc,
            "cos_t": cos_t,
            "sin_t": sin_t,
            "masks": masks,
            "ident": ident,
            "ones": ones,
        })
    return in_maps


def assemble_output(results):
    outT = np.concatenate([results[i]["outT"] for i in range(N_CORES)], axis=0)
    return np.ascontiguousarray(outT.T).reshape(B, S, C).astype(np.float32)


def kernel(x, Wqkv, Wproj):
    nc = _get_nc()
    in_maps = make_in_maps(x, Wqkv, Wproj)
    res = run_bass_kernel_spmd(nc, in_maps, core_ids=list(range(N_CORES)))
    return assemble_output(res.results)
